# revision 19
# baseline (speedup 1.0000x reference)
"""DiffVolume Trainium2 kernel.

volume[b, c, d, h, w] = left[b, c, h, w] - right[b, c, h, w - d]  (0 where w < d)

Shapes (hardcoded): left/right (2, 32, 96, 320) f32, D = 48.
Sharding: flatten (b, c) -> bc = 64, shard bc across 8 cores (8 bc each).
Each core reads its (8, 96, 320) input shard and writes its (48, 8, 96, 320)
output chunk (d-major); the host reorders to bc-major, concatenates, and
upcasts to f32.

The kernel is HBM-write-bound (output is 24x the input), so the wire format
is float16: inputs are rounded to f16 on the host, subs run on DVE in f16
(2x DVE mode), and the f16 volume is upcast on the host after the gather.
Worst-case elementwise error is ~3 ulp_f16 * max|x| ~ 1.2e-2 absolute /
~1.5e-3 relative to max|volume| -- far inside the 2e-2 gate.

Measured on HW: partial-width row writes (w >= d slices, 552-640B runs)
sink HBM write efficiency to ~220 GB/s, while full-W contiguous plane
writes reach ~400 GB/s. So the whole per-core volume lives in SBUF
(48d x 6t x 320w x f16 = 180KiB per partition), the w < d triangle is
zeroed once up front (Pool-engine rectangle memsets, off the DVE/DMA
critical path), and every output DMA moves full-W planes:
one DMA per (12-disparity group, 128-row block) = 24 DMAs
(HWDGE is a serial 625ns/DMA resource, so few fat DMAs).

Per-core layout:
 - 768 rows (bc, h) -> 6 blocks of 128 partitions (row r = t*128 + p).
 - left/right resident in SBUF as [128, 6*320] f16, loaded in 4 DMAs on the
   Activation queue (blocks 0-1 first so compute starts early; separate
   queue so input loads never head-block output DMAs on SP).
 - Subs are per-(d, block-pair): DMA(group, t) depends on ~2 blocks of DVE
   work, close to its own transfer time -- a barrier-free pipeline -- while
   halving the per-instruction DVE overhead vs per-block subs.
"""

import numpy as np

MAX_DISP = 48
B, C, H, W = 2, 32, 96, 320
NCORES = 8
BC = B * C                 # 64
BC_PER = BC // NCORES      # 8 bc rows per core
ROWS = BC_PER * H          # 768
P = 128
NT = ROWS // P             # 6 row blocks
DG = 6                     # disparities per output DMA group
NG = MAX_DISP // DG        # 8 groups

_NC_CACHE = {}


def build_body(nc, tc, left, right, out, rep=1):
    """Emit the kernel body. rep>1 re-runs the sub+DMA loop (for benchmarks)."""
    import concourse.mybir as mybir

    f16 = mybir.dt.float16
    # out viewed with (bc h) merged: [D, 768 rows, W]
    o_rows = out[:].rearrange("d bc h w -> d (bc h) w")
    with tc.tile_pool(name="io", bufs=1) as iop:
        lt = iop.tile([P, NT * W], f16)
        rt = iop.tile([P, NT * W], f16)
        vt = iop.tile([P, MAX_DISP * NT * W], f16)  # whole volume, resident
        l3 = lt[:].rearrange("p (t w) -> p t w", t=NT, w=W)
        r3 = rt[:].rearrange("p (t w) -> p t w", t=NT, w=W)
        o4 = vt[:].rearrange("p (d t w) -> p d t w", d=MAX_DISP, t=NT, w=W)
        lsrc = left[:].rearrange("bc h w -> (bc h) w").rearrange(
            "(t p) w -> p t w", p=P
        )
        rsrc = right[:].rearrange("bc h w -> (bc h) w").rearrange(
            "(t p) w -> p t w", p=P
        )
        # Zero the w < d region once: per 12-group, one rectangle memset
        # covering w < d0+DG for all its disparities (subs overwrite the
        # w >= d part). Pool engine: overlaps the input loads, touches
        # neither DVE nor the DMA engines.
        for gi in range(NG):
            d0 = gi * DG
            nc.gpsimd.memset(o4[:, d0 : d0 + DG, :, 0 : d0 + DG], 0.0)

        # Input loads: blocks 0-1 first (unblock the first t-pair subs), then
        # blocks 2-5 in one DMA per tensor, on the Activation queue.
        nc.scalar.dma_start(out=l3[:, 0:2, :], in_=lsrc[:, 0:2, :])
        nc.scalar.dma_start(out=r3[:, 0:2, :], in_=rsrc[:, 0:2, :])
        nc.scalar.dma_start(out=l3[:, 2:NT, :], in_=lsrc[:, 2:NT, :])
        nc.scalar.dma_start(out=r3[:, 2:NT, :], in_=rsrc[:, 2:NT, :])

        for _ in range(rep):
            for gi in range(NG):
                d0 = gi * DG
                # one sub per disparity covering all 6 blocks -- minimal
                # per-instruction DVE overhead; the SBUF-resident volume's
                # DMA backlog hides the group-completion barrier
                for j in range(DG):
                    d = d0 + j
                    nc.vector.tensor_sub(
                        o4[:, d, :, d:W],
                        l3[:, :, d:W],
                        r3[:, :, 0 : W - d],
                    )
                for t in range(NT):
                    dest = o_rows[
                        d0 : d0 + DG, t * P : (t + 1) * P, :
                    ].rearrange("d r w -> r d w")
                    # alternate the issuing queue so DMA dispatch is not
                    # serialized behind one sequencer's per-DMA issue cost
                    eng = nc.sync if t % 2 == 0 else nc.scalar
                    eng.dma_start(out=dest, in_=o4[:, d0 : d0 + DG, t, :])


def _build_nc(rep=1):
    import concourse.bacc as bacc
    import concourse.mybir as mybir
    from concourse import tile

    f16 = mybir.dt.float16
    nc = bacc.Bacc("TRN2")
    left = nc.dram_tensor("left", [BC_PER, H, W], f16, kind="ExternalInput")
    right = nc.dram_tensor("right", [BC_PER, H, W], f16, kind="ExternalInput")
    out = nc.dram_tensor(
        "out", [MAX_DISP, BC_PER, H, W], f16, kind="ExternalOutput"
    )

    with tile.TileContext(nc) as tc:
        build_body(nc, tc, left, right, out, rep=rep)
    nc.finalize()
    return nc


def _get_nc():
    if "nc" not in _NC_CACHE:
        _NC_CACHE["nc"] = _build_nc()
    return _NC_CACHE["nc"]


def make_in_maps(left_feature, right_feature):
    """Per-core input dicts (f16 wire format), bc-sharded."""
    lf = np.asarray(left_feature).astype(np.float16).reshape(BC, H, W)
    rf = np.asarray(right_feature).astype(np.float16).reshape(BC, H, W)
    return [
        {
            "left": np.ascontiguousarray(lf[k * BC_PER : (k + 1) * BC_PER]),
            "right": np.ascontiguousarray(rf[k * BC_PER : (k + 1) * BC_PER]),
        }
        for k in range(NCORES)
    ]


def run(left_feature, right_feature, **spmd_kwargs):
    """Run the SPMD kernel; returns (volume, BassKernelResults)."""
    from concourse.bass_utils import run_bass_kernel_spmd

    nc = _get_nc()
    in_maps = make_in_maps(left_feature, right_feature)
    res = run_bass_kernel_spmd(nc, in_maps, core_ids=list(range(NCORES)), **spmd_kwargs)
    # per-core chunks are [D, BC_PER, H, W] f16; concat bc, reorder d <-> bc
    chunks = [res.results[k]["out"] for k in range(NCORES)]
    vol = (
        np.concatenate(chunks, axis=1)
        .transpose(1, 0, 2, 3)
        .reshape(B, C, MAX_DISP, H, W)
        .astype(np.float32)
    )
    return vol, res


def kernel(left_feature, right_feature):
    vol, _ = run(left_feature, right_feature)
    return vol


# revision 24
# speedup vs baseline: 1.2323x; 1.2323x over previous
"""DiffVolume Trainium2 kernel.

volume[b, c, d, h, w] = left[b, c, h, w] - right[b, c, h, w - d]  (0 where w < d)

Shapes (hardcoded): left/right (2, 32, 96, 320) f32, D = 48.
Sharding: flatten (b, c) -> bc = 64, shard bc across 8 cores (8 bc each).
Each core reads its (8, 96, 320) input shard and writes its (48, 8, 96, 320)
output chunk (d-major); the host reorders to bc-major, concatenates, and
upcasts to f32.

The kernel is HBM-write-bound (output is 24x the input), so the wire format
is float16: inputs are rounded to f16 on the host, subs run on DVE in f16
(2x DVE mode), and the f16 volume is upcast on the host after the gather.
Worst-case elementwise error is ~3 ulp_f16 * max|x| ~ 1.2e-2 absolute /
~1.5e-3 relative to max|volume| -- far inside the 2e-2 gate.

Measured on HW: partial-width row writes (w >= d slices, 552-640B runs)
sink HBM write efficiency to ~220 GB/s, while full-W contiguous plane
writes reach ~400 GB/s. So the whole per-core volume lives in SBUF
(48d x 6t x 320w x f16 = 180KiB per partition), the w < d triangle is
zeroed once up front (Pool-engine rectangle memsets, off the DVE/DMA
critical path), and every output DMA moves full-W planes:
one DMA per (6-disparity group, 128-row block) = 48 DMAs, each ~0.5MB
(HWDGE's serial per-DMA cost stays well under the transfer time).

Per-core layout:
 - 768 rows (bc, h) -> 6 blocks of 128 partitions (row r = t*128 + p).
 - left/right resident in SBUF as [128, 6*320] f16, loaded in 4 DMAs on the
   Activation queue (blocks 0-1 first so compute starts early; separate
   queue so input loads never head-block output DMAs on SP).
 - Group 0's subs are per-(d, block-pair) so the first DMAs depend on only
   ~2 blocks of DVE work (short ramp); steady-state groups use one sub per
   disparity covering all 6 blocks (minimal per-instruction DVE overhead,
   which real HW charges more heavily than the cost model), with the
   SBUF-resident backlog hiding the 6-sub group barrier.
"""

import numpy as np

MAX_DISP = 48
B, C, H, W = 2, 32, 96, 320
NCORES = 8
BC = B * C                 # 64
BC_PER = BC // NCORES      # 8 bc rows per core
ROWS = BC_PER * H          # 768
P = 128
NT = ROWS // P             # 6 row blocks
DG = 6                     # disparities per output DMA group
NG = MAX_DISP // DG        # 8 groups

_NC_CACHE = {}


def build_body(nc, tc, left, right, out, rep=1):
    """Emit the kernel body. rep>1 re-runs the sub+DMA loop (for benchmarks)."""
    import concourse.mybir as mybir

    f16 = mybir.dt.float16
    # out viewed with (bc h) merged: [D, 768 rows, W]
    o_rows = out[:].rearrange("d bc h w -> d (bc h) w")
    with tc.tile_pool(name="io", bufs=1) as iop:
        lt = iop.tile([P, NT * W], f16)
        rt = iop.tile([P, NT * W], f16)
        vt = iop.tile([P, MAX_DISP * NT * W], f16)  # whole volume, resident
        l3 = lt[:].rearrange("p (t w) -> p t w", t=NT, w=W)
        r3 = rt[:].rearrange("p (t w) -> p t w", t=NT, w=W)
        o4 = vt[:].rearrange("p (d t w) -> p d t w", d=MAX_DISP, t=NT, w=W)
        lsrc = left[:].rearrange("bc h w -> (bc h) w").rearrange(
            "(t p) w -> p t w", p=P
        )
        rsrc = right[:].rearrange("bc h w -> (bc h) w").rearrange(
            "(t p) w -> p t w", p=P
        )
        # Zero the w < d region once: per 12-group, one rectangle memset
        # covering w < d0+DG for all its disparities (subs overwrite the
        # w >= d part). Pool engine: overlaps the input loads, touches
        # neither DVE nor the DMA engines.
        for gi in range(NG):
            d0 = gi * DG
            nc.gpsimd.memset(o4[:, d0 : d0 + DG, :, 0 : d0 + DG], 0.0)

        # Input loads: blocks 0-1 first (unblock the first t-pair subs), then
        # blocks 2-5 in one DMA per tensor, on the Activation queue.
        nc.scalar.dma_start(out=l3[:, 0:2, :], in_=lsrc[:, 0:2, :])
        nc.scalar.dma_start(out=r3[:, 0:2, :], in_=rsrc[:, 0:2, :])
        nc.scalar.dma_start(out=l3[:, 2:NT, :], in_=lsrc[:, 2:NT, :])
        nc.scalar.dma_start(out=r3[:, 2:NT, :], in_=rsrc[:, 2:NT, :])

        for _ in range(rep):
            for gi in range(NG):
                d0 = gi * DG
                if gi == 0:
                    # leading group: subs per (d, block-pair) so the first
                    # DMAs depend on ~2 blocks of DVE work (short ramp)
                    for t in range(0, NT, 2):
                        for j in range(DG):
                            d = d0 + j
                            nc.vector.tensor_sub(
                                o4[:, d, t : t + 2, d:W],
                                l3[:, t : t + 2, d:W],
                                r3[:, t : t + 2, 0 : W - d],
                            )
                        for tt in (t, t + 1):
                            dest = o_rows[
                                d0 : d0 + DG, tt * P : (tt + 1) * P, :
                            ].rearrange("d r w -> r d w")
                            nc.sync.dma_start(
                                out=dest, in_=o4[:, d0 : d0 + DG, tt, :]
                            )
                else:
                    # steady state: one sub per disparity covering all 6
                    # blocks -- minimal per-instruction DVE overhead; the
                    # DMA backlog hides the group-completion barrier
                    for j in range(DG):
                        d = d0 + j
                        nc.vector.tensor_sub(
                            o4[:, d, :, d:W],
                            l3[:, :, d:W],
                            r3[:, :, 0 : W - d],
                        )
                    for t in range(NT):
                        dest = o_rows[
                            d0 : d0 + DG, t * P : (t + 1) * P, :
                        ].rearrange("d r w -> r d w")
                        nc.sync.dma_start(
                            out=dest, in_=o4[:, d0 : d0 + DG, t, :]
                        )


def _build_nc(rep=1):
    import concourse.bacc as bacc
    import concourse.mybir as mybir
    from concourse import tile

    f16 = mybir.dt.float16
    nc = bacc.Bacc("TRN2")
    left = nc.dram_tensor("left", [BC_PER, H, W], f16, kind="ExternalInput")
    right = nc.dram_tensor("right", [BC_PER, H, W], f16, kind="ExternalInput")
    out = nc.dram_tensor(
        "out", [MAX_DISP, BC_PER, H, W], f16, kind="ExternalOutput"
    )

    with tile.TileContext(nc) as tc:
        build_body(nc, tc, left, right, out, rep=rep)
    nc.finalize()
    return nc


def _get_nc():
    if "nc" not in _NC_CACHE:
        _NC_CACHE["nc"] = _build_nc()
    return _NC_CACHE["nc"]


def make_in_maps(left_feature, right_feature):
    """Per-core input dicts (f16 wire format), bc-sharded."""
    lf = np.asarray(left_feature).astype(np.float16).reshape(BC, H, W)
    rf = np.asarray(right_feature).astype(np.float16).reshape(BC, H, W)
    return [
        {
            "left": np.ascontiguousarray(lf[k * BC_PER : (k + 1) * BC_PER]),
            "right": np.ascontiguousarray(rf[k * BC_PER : (k + 1) * BC_PER]),
        }
        for k in range(NCORES)
    ]


def run(left_feature, right_feature, **spmd_kwargs):
    """Run the SPMD kernel; returns (volume, BassKernelResults)."""
    from concourse.bass_utils import run_bass_kernel_spmd

    nc = _get_nc()
    in_maps = make_in_maps(left_feature, right_feature)
    res = run_bass_kernel_spmd(nc, in_maps, core_ids=list(range(NCORES)), **spmd_kwargs)
    # per-core chunks are [D, BC_PER, H, W] f16; concat bc, reorder d <-> bc
    chunks = [res.results[k]["out"] for k in range(NCORES)]
    vol = (
        np.concatenate(chunks, axis=1)
        .transpose(1, 0, 2, 3)
        .reshape(B, C, MAX_DISP, H, W)
        .astype(np.float32)
    )
    return vol, res


def kernel(left_feature, right_feature):
    vol, _ = run(left_feature, right_feature)
    return vol
